# revision 28
# baseline (speedup 1.0000x reference)
"""GCN layer kernel for Trainium2 (8 NeuronCores, SPMD).

out = relu( D^{-1/2} (A+I) D^{-1/2} x W^T + b )

Math restructure (projection commutes with aggregation):
    out[i] = relu( dinv[i] * (sum_{(i,j) in E+self} xp[j]) @ W^T + b )
with xp = dinv[:,None] * x  (host-prescaled, bf16).

Single-phase device plan per core (core c owns src rows [c*6250,(c+1)*6250)):
  For each 128-src-node chunk:
    - dma_gather xp[dst] rows for the chunk's non-self edges (host-bucketed,
      int16 indices, lo/hi windows for the 32768 int16 limit). Gathers are
      round-robined over 4 SWDGE queues so 4 Q7 core pairs emit descriptors
      concurrently (single-queue emission was the old kernel's bottleneck:
      98 gathers x 10us serialized on one Q7 pair = 984us of 1199us).
    - self-loop rows are a contiguous 128-row block of the core's own shard:
      plain HWDGE load from a per-core xself input; aggregated via an
      identity one-hot (no gather descriptors for them).
    - segment-sum via one-hot matmuls in f-major psum: ps[f,i] += G_half x S.
      S matrices are built on DVE in batches of 8 blocks per instruction.
    - per-chunk projection ps2[i,o] = aggT halves x W^T halves; bias added
      in-psum by a K=1 outer-product matmul (sqrt(deg)_i x b_o); one fused
      ACT epilogue relu(dinv_i * ps2) -> staged output rows.

Host does only sharding/layout work: edge bucketing, int16 index packing,
degree counting, scaling/casts.
"""

import sys

for _p in ("/opt/trn_rl_repo",):
    if _p not in sys.path:
        sys.path.insert(0, _p)

from contextlib import ExitStack

import ml_dtypes
import numpy as np

import concourse.bass as bass
import concourse.mybir as mybir
import concourse.tile as tile
from concourse import bacc
from concourse.bass_utils import run_bass_kernel_spmd

BF16 = ml_dtypes.bfloat16

N_NODES = 50000
N_EDGES = 800000
F = 256  # in_size == out_size == 256
N_CORES = 8
NPC = N_NODES // N_CORES  # 6250 nodes per core
SPLIT = 32768  # int16 index limit for dma_gather
CHUNKS = (NPC + 127) // 128  # 49 chunks of <=128 src nodes per core
NSELF = CHUNKS * 128  # 6272 padded self rows per core
OUT_GRP = 8  # output chunks per DRAM write
S_BATCH = 8  # one-hot blocks built per DVE instruction
N_QUEUES = 4  # SWDGE queues (4 Q7 core pairs emit gather descriptors in parallel)


def _pack_idx(vals, blocks):
    """int16 gather index layout: position i -> [i % 16, i // 16],
    replicated to 128 partitions. Pad is 0 (gathers row 0, masked to zero by
    the one-hot S whose pad slot is 200). NOTE: -1 trailing pads would let
    the ucode skip pad descriptors, but then num_idxs_reg must be a runtime
    register holding the exact per-core valid count (else the decode's ring
    reservation diverges from emission and stale descriptors fire); the 98
    value_load registers needed for that blow the Pool register budget."""
    n = blocks * 128
    a = np.zeros(n, dtype=np.int16)
    a[: len(vals)] = vals
    cols = a.reshape(n // 16, 16).T  # [16, n/16]
    return np.tile(cols, (8, 1))  # [128, n/16]


def _pack_slots(vals, blocks, pad_val=200.0):
    """slot layout: position i -> [i % 128, i // 128]."""
    n = blocks * 128
    a = np.full(n, pad_val, dtype=np.float32)
    a[: len(vals)] = vals
    return a.reshape(blocks, 128).T.astype(BF16)  # [128, blocks]


def _build_program(lo_blk, hi_blk):
    """Build the (core-uniform) Bass program. lo_blk/hi_blk: per-chunk
    gather block counts (lists of CHUNKS ints)."""
    nc = bacc.Bacc(
        None, target_bir_lowering=False, debug=False, num_swdge_queues=N_QUEUES
    )
    dt = mybir.dt

    sum_lo = int(sum(lo_blk))
    sum_hi = int(sum(hi_blk))
    sum_nb = sum_lo + sum_hi

    xp = nc.dram_tensor("xp", [N_NODES, F], dt.bfloat16, kind="ExternalInput")
    xself = nc.dram_tensor("xself", [NSELF, F], dt.bfloat16, kind="ExternalInput")
    wT = nc.dram_tensor("wt", [2, 128, F], dt.bfloat16, kind="ExternalInput")
    iota8 = nc.dram_tensor(
        "iota8", [128, S_BATCH, 128], dt.bfloat16, kind="ExternalInput"
    )
    ident = nc.dram_tensor("ident", [128, 128], dt.bfloat16, kind="ExternalInput")
    bias_row = nc.dram_tensor("bias_row", [1, F], dt.bfloat16, kind="ExternalInput")
    recip_row = nc.dram_tensor(
        "recip_row", [1, CHUNKS * 128], dt.bfloat16, kind="ExternalInput"
    )
    dinv_chk = nc.dram_tensor(
        "dinv_chk", [128, CHUNKS], dt.float32, kind="ExternalInput"
    )
    idx_lo = nc.dram_tensor("idx_lo", [128, 8 * sum_lo], dt.int16, kind="ExternalInput")
    idx_hi = nc.dram_tensor("idx_hi", [128, 8 * sum_hi], dt.int16, kind="ExternalInput")
    slots = nc.dram_tensor("slots", [128, sum_nb], dt.bfloat16, kind="ExternalInput")
    out = nc.dram_tensor("out", [NPC, F], dt.bfloat16, kind="ExternalOutput")

    with tile.TileContext(nc) as tc, ExitStack() as top:
        cpool = top.enter_context(tc.tile_pool(name="const", bufs=1))
        wt_s = cpool.tile([128, 2, F], dt.bfloat16)
        nc.sync.dma_start(out=wt_s[:, 0, :], in_=wT[0])
        nc.sync.dma_start(out=wt_s[:, 1, :], in_=wT[1])
        iota_s = cpool.tile([128, S_BATCH, 128], dt.bfloat16)
        nc.sync.dma_start(out=iota_s[:], in_=iota8[:])
        id_s = cpool.tile([128, 128], dt.bfloat16)
        nc.sync.dma_start(out=id_s[:], in_=ident[:])
        brow_s = cpool.tile([1, F], dt.bfloat16)
        nc.sync.dma_start(out=brow_s[:], in_=bias_row[:])
        rrow_s = cpool.tile([1, CHUNKS * 128], dt.bfloat16)
        nc.sync.dma_start(out=rrow_s[:], in_=recip_row[:])
        dvc_s = cpool.tile([128, CHUNKS], dt.float32)
        nc.sync.dma_start(out=dvc_s[:], in_=dinv_chk[:])
        # split the big idx-table loads so the first chunks' gathers can
        # start as soon as their slice has landed
        ilo_s = cpool.tile([128, 8 * sum_lo], dt.int16)
        for q0 in range(0, 8 * sum_lo, 8 * sum_lo // 4 + 8):
            q1 = min(8 * sum_lo, q0 + 8 * sum_lo // 4 + 8)
            nc.sync.dma_start(out=ilo_s[:, q0:q1], in_=idx_lo[:, q0:q1])
        ihi_s = cpool.tile([128, 8 * sum_hi], dt.int16)
        for q0 in range(0, 8 * sum_hi, 8 * sum_hi // 4 + 8):
            q1 = min(8 * sum_hi, q0 + 8 * sum_hi // 4 + 8)
            nc.sync.dma_start(out=ihi_s[:, q0:q1], in_=idx_hi[:, q0:q1])
        slt_s = cpool.tile([128, sum_nb], dt.bfloat16)
        nc.sync.dma_start(out=slt_s[:], in_=slots[:])

        with ExitStack() as p2:
            gpool = p2.enter_context(tc.tile_pool(name="gat", bufs=10))
            sfpool = p2.enter_context(tc.tile_pool(name="gself", bufs=4))
            spool = p2.enter_context(tc.tile_pool(name="sel", bufs=6))
            apool = p2.enter_context(tc.tile_pool(name="aggt", bufs=3))
            opool = p2.enter_context(tc.tile_pool(name="ostg", bufs=3))
            # one accumulation chain per 2KB psum bank (zero-region rule):
            # psA and psB chains interleave, so they live in separate
            # bank-sized tiles; sequential chains may share a bank.
            pa_pool = p2.enter_context(tc.tile_pool(name="pa", bufs=2, space="PSUM"))
            pb_pool = p2.enter_context(tc.tile_pool(name="pb", bufs=2, space="PSUM"))
            po_pool = p2.enter_context(tc.tile_pool(name="po", bufs=2, space="PSUM"))

            lo_off = 0
            hi_off = 0
            nb_off = 0
            ob = None
            ob_base = 0
            og = 0
            for k in range(CHUNKS):
                LO, HI = int(lo_blk[k]), int(hi_blk[k])
                NB = LO + HI
                if ob is None:
                    og = min(OUT_GRP, CHUNKS - k)
                    ob = opool.tile([128, og, F], dt.bfloat16, tag="ob")
                    ob_base = k

                # ---- gather non-self edge rows (4-queue round-robin) ------
                G = gpool.tile([128, NB, F], dt.bfloat16, tag="G")
                nc.gpsimd.dma_gather(
                    G[:, 0:LO, :],
                    xp[0:SPLIT, :],
                    ilo_s[:, 8 * lo_off : 8 * (lo_off + LO)],
                    128 * LO,
                    128 * LO,
                    F,
                    single_packet=False,
                    # lo gathers are ~2x bigger than hi: spread each kind
                    # across all 4 queues so per-queue load balances
                    queue_num=k % N_QUEUES,
                )
                nc.gpsimd.dma_gather(
                    G[:, LO:NB, :],
                    xp[SPLIT:N_NODES, :],
                    ihi_s[:, 8 * hi_off : 8 * (hi_off + HI)],
                    128 * HI,
                    128 * HI,
                    F,
                    single_packet=False,
                    queue_num=(k + 2) % N_QUEUES,
                )

                # ---- self-loop rows: contiguous HWDGE load ----------------
                gs = sfpool.tile([128, F], dt.bfloat16, tag="gs")
                nc.sync.dma_start(out=gs[:], in_=xself[128 * k : 128 * (k + 1), :])

                # ---- one-hot S builds, batched ----------------------------
                s_tiles = []
                for s0 in range(0, NB, S_BATCH):
                    sb = min(S_BATCH, NB - s0)
                    S = spool.tile([128, sb, 128], dt.bfloat16, tag="S")
                    nc.vector.tensor_tensor(
                        out=S[:],
                        in0=slt_s[:, nb_off + s0 : nb_off + s0 + sb].to_broadcast(
                            [128, sb, 128]
                        ),
                        in1=iota_s[:, 0:sb, :],
                        op=mybir.AluOpType.is_equal,
                    )
                    s_tiles.append((s0, sb, S))

                # ---- segment-sum matmuls (f-major psum) -------------------
                psa = pa_pool.tile([128, 512], dt.float32)  # full bank
                psb = pb_pool.tile([128, 512], dt.float32)  # full bank
                for s0, sb, S in s_tiles:
                    for j in range(sb):
                        b = s0 + j
                        first = b == 0
                        nc.tensor.matmul(
                            out=psa[:, 0:128],
                            lhsT=G[:, b, 0:128],
                            rhs=S[:, j, :],
                            start=first,
                            stop=False,
                        )
                        nc.tensor.matmul(
                            out=psb[:, 0:128],
                            lhsT=G[:, b, 128:256],
                            rhs=S[:, j, :],
                            start=first,
                            stop=False,
                        )
                # self-loop contribution closes the accumulation
                nc.tensor.matmul(
                    out=psa[:, 0:128],
                    lhsT=gs[:, 0:128],
                    rhs=id_s[:],
                    start=False,
                    stop=True,
                )
                nc.tensor.matmul(
                    out=psb[:, 0:128],
                    lhsT=gs[:, 128:256],
                    rhs=id_s[:],
                    start=False,
                    stop=True,
                )

                # ---- psum -> sbuf (bf16) ----------------------------------
                at = apool.tile([128, 2, 128], dt.bfloat16, tag="at")
                nc.scalar.activation(
                    out=at[:, 0, :],
                    in_=psa[:, 0:128],
                    func=mybir.ActivationFunctionType.Copy,
                )
                nc.scalar.activation(
                    out=at[:, 1, :],
                    in_=psb[:, 0:128],
                    func=mybir.ActivationFunctionType.Copy,
                )

                # ---- projection + in-psum bias ----------------------------
                pot = po_pool.tile([128, 512], dt.float32)  # full bank
                po = pot[:, 0:F]
                nc.tensor.matmul(
                    out=po[:],
                    lhsT=at[:, 0, :],
                    rhs=wt_s[:, 0, :],
                    start=True,
                    stop=False,
                )
                nc.tensor.matmul(
                    out=po[:],
                    lhsT=at[:, 1, :],
                    rhs=wt_s[:, 1, :],
                    start=False,
                    stop=False,
                )
                nc.tensor.matmul(
                    out=po[:],
                    lhsT=rrow_s[:, 128 * k : 128 * (k + 1)],
                    rhs=brow_s[:],
                    start=False,
                    stop=True,
                )

                # ---- fused epilogue: relu(dinv * po) ----------------------
                nc.scalar.activation(
                    out=ob[:, k - ob_base, :],
                    in_=po[:],
                    func=mybir.ActivationFunctionType.Relu,
                    scale=dvc_s[:, k : k + 1],
                )

                if k - ob_base + 1 == og:
                    r0 = ob_base * 128
                    rw = og * 128
                    if r0 + rw <= NPC:
                        dst = out[r0 : r0 + rw, :].rearrange("(t p) f -> p t f", p=128)
                        nc.sync.dma_start(out=dst, in_=ob[:])
                    else:
                        # tail group: full chunks + one partial (106 rows)
                        full = (NPC - r0) // 128
                        if full:
                            dst = out[r0 : r0 + full * 128, :].rearrange(
                                "(t p) f -> p t f", p=128
                            )
                            nc.sync.dma_start(out=dst, in_=ob[:, :full, :])
                        rem = NPC - r0 - full * 128
                        if rem:
                            nc.sync.dma_start(
                                out=out[r0 + full * 128 : NPC, :],
                                in_=ob[:rem, full, :],
                            )
                    ob = None
                lo_off += LO
                hi_off += HI
                nb_off += NB

    nc.compile()
    return nc


def _prep(x, edge_index, W, b):
    """Host-side sharding/layout. Returns (lo_blk, hi_blk, common, per_core)."""
    src = np.asarray(edge_index[0], dtype=np.int64)
    dst = np.asarray(edge_index[1], dtype=np.int64)
    deg = np.bincount(src, minlength=N_NODES).astype(np.float32)
    dinv = deg**-0.5

    # pre-scaled node features (bf16) — the only tensor the device gathers
    xp = (np.asarray(x, dtype=np.float32) * dinv[:, None]).astype(BF16)

    # bucket NON-SELF edges by (core, chunk, lo/hi window)
    core = src // NPC
    src_local = src - core * NPC
    chunk = src_local >> 7
    slot = src_local & 127
    is_hi = (dst >= SPLIT).astype(np.int64)
    key = (core * CHUNKS + chunk) * 2 + is_hi
    order = np.argsort(key, kind="stable")
    key_s = key[order]
    dst_s = dst[order]
    slot_s = slot[order]

    nseg = N_CORES * CHUNKS * 2
    counts = np.bincount(key_s, minlength=nseg).reshape(N_CORES, CHUNKS, 2)
    seg_end = np.cumsum(counts.reshape(-1))
    seg_start = seg_end - counts.reshape(-1)

    lo_max = counts[:, :, 0].max(axis=0)  # [CHUNKS]
    hi_max = counts[:, :, 1].max(axis=0)
    lo_blk = np.maximum(1, (lo_max + 127) // 128).astype(np.int64)
    hi_blk = np.maximum(1, (hi_max + 127) // 128).astype(np.int64)

    # common (replicated) tensors
    wT = np.ascontiguousarray(np.asarray(W, dtype=np.float32).T).astype(BF16)
    wt_in = np.stack([wT[:128], wT[128:]])  # [2,128,F]
    iota_t = np.tile(
        np.arange(128, dtype=np.float32)[None, None, :], (128, S_BATCH, 1)
    ).astype(BF16)
    ident = np.eye(128, dtype=np.float32).astype(BF16)
    bias_row = np.asarray(b, dtype=np.float32)[None, :].astype(BF16)
    common = dict(xp=xp, wt=wt_in, iota8=iota_t, ident=ident, bias_row=bias_row)

    sum_lo = int(lo_blk.sum())
    sum_hi = int(hi_blk.sum())
    per_core = []
    for c in range(N_CORES):
        ilo = np.zeros((128, 8 * sum_lo), dtype=np.int16)
        ihi = np.zeros((128, 8 * sum_hi), dtype=np.int16)
        slt = np.zeros((128, sum_lo + sum_hi), dtype=BF16)
        gcnt = np.zeros((1, 2 * CHUNKS), dtype=np.int32)
        lo_off = hi_off = nb_off = 0
        for k in range(CHUNKS):
            LO, HI = int(lo_blk[k]), int(hi_blk[k])
            s = (c * CHUNKS + k) * 2
            a0, a1 = seg_start[s], seg_end[s]
            b0, b1 = seg_start[s + 1], seg_end[s + 1]
            gcnt[0, 2 * k] = a1 - a0
            gcnt[0, 2 * k + 1] = b1 - b0
            ilo[:, 8 * lo_off : 8 * (lo_off + LO)] = _pack_idx(dst_s[a0:a1], LO)
            ihi[:, 8 * hi_off : 8 * (hi_off + HI)] = _pack_idx(
                dst_s[b0:b1] - SPLIT, HI
            )
            slt[:, nb_off : nb_off + LO] = _pack_slots(slot_s[a0:a1], LO)
            slt[:, nb_off + LO : nb_off + LO + HI] = _pack_slots(slot_s[b0:b1], HI)
            lo_off += LO
            hi_off += HI
            nb_off += LO + HI

        # per-core self rows (padded to 6272 with zeros)
        xself = np.zeros((NSELF, F), dtype=BF16)
        xself[:NPC] = xp[c * NPC : (c + 1) * NPC]

        # dinv per (slot, chunk) for the epilogue scale
        nchk = np.arange(128)[:, None] + 128 * np.arange(CHUNKS)[None, :] + c * NPC
        dvc = np.where(
            nchk - c * NPC < NPC, dinv[np.minimum(nchk, N_NODES - 1)], 1.0
        ).astype(np.float32)

        # sqrt(deg) per chunk-slot for the in-psum bias outer product
        rr = np.ones(CHUNKS * 128, dtype=np.float32)
        valid = np.arange(CHUNKS * 128) < NPC
        rr[valid] = np.sqrt(deg[c * NPC : (c + 1) * NPC])
        recip_row = rr[None, :].astype(BF16)

        per_core.append(
            dict(
                idx_lo=ilo,
                idx_hi=ihi,
                slots=slt,
                dinv_chk=np.ascontiguousarray(dvc),
                recip_row=recip_row,
                xself=xself,
            )
        )
    return lo_blk, hi_blk, common, per_core


def _install_ntff_hook():
    """The agent image's antenv lacks axon_hooks; recreate it so
    run_bass_kernel_spmd(trace=True) can profile via the axon .so."""
    import types

    if "antenv.axon_hooks" in sys.modules:
        return
    mod = types.ModuleType("antenv.axon_hooks")
    state = {}
    mod.set_axon_ntff_profile_hook = lambda h: state.__setitem__("h", h)
    mod.get_axon_ntff_profile_hook = lambda: state.get("h")
    sys.modules["antenv.axon_hooks"] = mod
    try:
        import antenv

        antenv.axon_hooks = mod
    except Exception:
        pass
    try:
        if "/root/.axon_site" not in sys.path:
            sys.path.insert(0, "/root/.axon_site")
        from trn_agent_boot.trn_boot import _ntff_profile_via_ctypes

        mod.set_axon_ntff_profile_hook(
            _ntff_profile_via_ctypes("/opt/axon/libaxon_pjrt.so")
        )
    except Exception:
        pass


_CACHE = {}


def kernel(x, edge_index, W, b, trace=False):
    if trace:
        _install_ntff_hook()
    lo_blk, hi_blk, common, per_core = _prep(x, edge_index, W, b)
    key = (tuple(lo_blk), tuple(hi_blk))
    if key not in _CACHE:
        _CACHE[key] = _build_program(lo_blk, hi_blk)
    nc = _CACHE[key]

    in_maps = []
    for c in range(N_CORES):
        m = dict(common)
        m.update(per_core[c])
        in_maps.append(m)

    res = run_bass_kernel_spmd(
        nc, in_maps, core_ids=list(range(N_CORES)), trace=trace
    )
    out = np.concatenate([r["out"] for r in res.results], axis=0)
    if trace:
        kernel.last_exec_ns = res.exec_time_ns
        kernel.last_profile = res.profile_json
    return out.astype(np.float32)
